# revision 23
# baseline (speedup 1.0000x reference)
"""Trainium2 Bass kernel for nn_FFTCNN (FFT-conv text classifier).

Math: the reference's fft_conv1d (irfft(rfft(x) * rfft(w_pad))) is exactly a
K=3 circular convolution.  conv1 is linear in the embedding, so the host
precomputes the fused table EW[t, k*128+o] = (emb @ w1[:,:,k].T)[t, o] and
the device gathers conv1's output contributions directly.

Perf structure (vs the fp16 baseline):
  - conv2 runs as 2 fp8e4m3 DoubleRow matmuls per l-tile (contraction 256
    per instruction) instead of 3 fp16 matmuls.  DoubleRow pair windows
    must not overlap on HW, so h1 is dual-written as fp8 to two copies (A
    at col 0, B at col HD=L+3 of one buffer) by a single ScalarE
    activation with a broadcast input AP; the conv2 rhs pair AP strides
    HD+1 between the two k-plane windows.
  - Scales: h1 = 2^8*relu(conv1+b1), w2q = 2^12*w2; descaled 2^-20 at the
    pooling activation (max-pool commutes with the monotonic +b2/relu).
  - Each batch element's conv2 block is DELAYED into the middle of the
    next batch's conv1 phase, filling PE gather-wait gaps and shortening
    the post-last-gather tail.  Fine-grained conv1/conv2 interleaving
    regresses (DR<->normal switching + ScalarE coupling) - keep blocks.
  - Gather: 32 single-packet SWDGE gathers (4 queues) of 514 valid idxs;
    the gather ucode descriptor generation on the Pool DSPs is the pacing
    resource (~56us); chunk-size/multi-packet/fp8-row variants all
    measured slower (fp8 512B rows also slow conv1 via stride-2 rhs).

Sharding: data-parallel over batch - 8 cores x 4 batch elements.
"""

import os
import sys

sys.path.insert(0, "/opt/trn_rl_repo")

import numpy as np

B, L = 32, 4096
VOCAB, EMB, HID, CLASSES = 20000, 512, 128, 6
K = 3
NCORES = 8
BLOC = B // NCORES          # batch elements per core
LTILE = 512
NLT = L // LTILE            # 8 l-tiles
EWC = K * HID               # fused table row length (384)

HD = L + 3                  # h1 copy stride (odd: keeps window offsets even)
H1S = 256.0                 # h1 scale (2^8)
W2S = 4096.0                # w2 scale (2^12)

GCHUNK = int(os.environ.get("KERNEL_GCHUNK", "8"))
SINGLE_PACKET = os.environ.get("KERNEL_SINGLE_PACKET", "1") == "1"
NQUEUES = int(os.environ.get("KERNEL_NQUEUES", "4"))
XTBUFS = int(os.environ.get("KERNEL_XTBUFS", str(4 * GCHUNK)))
DUAL_WRITE = os.environ.get("KERNEL_DUAL_WRITE", "1") == "1"
CSPAN = L // GCHUNK
CNIDX = CSPAN + 128
NVALID = CSPAN + 2   # real indices per chunk; the rest are -1 (ucode trims)
# HW limit: the single-packet tx stream allows <=64 descriptors/engine,
# i.e. num_idxs/16 + 2 <= 64 -> num_idxs <= 992 per dma_gather.
assert not SINGLE_PACKET or CNIDX <= 992


def _dtype_np(mode):
    import ml_dtypes

    return np.float16 if mode == "f16" else ml_dtypes.bfloat16


def _dtype_my(mode):
    import concourse.mybir as mybir

    return mybir.dt.float16 if mode == "f16" else mybir.dt.bfloat16


def build_program(mode="f16", nbatch=BLOC):
    """Build the per-core Bass program."""
    import concourse.bacc as bacc
    import concourse.mybir as mybir
    import concourse.tile as tile
    from concourse._compat import get_trn_type
    from concourse.bass import AP

    f32 = mybir.dt.float32
    f16 = _dtype_my(mode)
    f8 = mybir.dt.float8e4
    u8 = mybir.dt.uint8
    i16 = mybir.dt.int16
    RELU = mybir.ActivationFunctionType.Relu
    IDENT = mybir.ActivationFunctionType.Identity
    COPY = mybir.ActivationFunctionType.Copy
    DR = mybir.MatmulPerfMode.DoubleRow
    AX = mybir.AxisListType.X

    nc = bacc.Bacc(
        get_trn_type() or "TRN2",
        target_bir_lowering=False,
        debug=False,
        enable_asserts=False,
        num_devices=NCORES,
        num_swdge_queues=NQUEUES,
        dynamic_dma_scratch_size=49152,
    )

    ncols = CNIDX // 16  # idx columns per gather chunk

    ew_d = nc.dram_tensor("ew", [VOCAB, EWC], f16, kind="ExternalInput")
    idx_d = nc.dram_tensor("idx", [128, nbatch * GCHUNK * ncols], i16,
                           kind="ExternalInput")
    id_d = nc.dram_tensor("id128", [128, 128], f16, kind="ExternalInput")
    # fp8 DoubleRow lhsT pairs, as raw bytes: (w2k2|w2k1) and (w2k0|0)
    w2p_d = nc.dram_tensor("w2p", [128, 2 * 256], u8, kind="ExternalInput")
    b1_d = nc.dram_tensor("b1c", [128, 1], f32, kind="ExternalInput")
    b2_d = nc.dram_tensor("b2c", [128, 1], f32, kind="ExternalInput")
    lw1_d = nc.dram_tensor("lw1t", [128, HID], f16, kind="ExternalInput")
    lb1_d = nc.dram_tensor("lb1c", [128, 1], f32, kind="ExternalInput")
    lw2_d = nc.dram_tensor("lw2t", [128, CLASSES], f16, kind="ExternalInput")
    lb2_d = nc.dram_tensor("lb2c", [CLASSES, 1], f32, kind="ExternalInput")
    out_d = nc.dram_tensor("out", [CLASSES, nbatch], f32, kind="ExternalOutput")

    from concourse import library_config

    def pair_ap(base, off, istride, n):
        """rhs AP [128, 2, n]: i-stride/n-stride in elements of base."""
        return AP(base.tensor, base.offset + off,
                  [base.ap[0], (istride, 2), (1, n)])

    with tile.TileContext(nc) as tc:
        nc.gpsimd.load_library(library_config.mlp)
        with (
            tc.tile_pool(name="const", bufs=1) as cpool,
            tc.tile_pool(name="xt", bufs=XTBUFS) as xt_pool,
            tc.tile_pool(name="h1", bufs=2) as h1_pool,
            tc.tile_pool(name="small", bufs=2) as sm_pool,
            tc.tile_pool(name="ps", bufs=6, space="PSUM") as ps_pool,
            tc.tile_pool(name="psm", bufs=1, space="PSUM") as psm_pool,
        ):
            id_sb = cpool.tile([128, 128], f16)
            nc.sync.dma_start(id_sb[:, :], id_d.ap())
            w2p_sb = cpool.tile([128, 2 * 256], u8)
            nc.sync.dma_start(w2p_sb[:, :], w2p_d.ap())
            w2p_f8 = w2p_sb[:, :].bitcast(f8)
            # lhsT [128, 2, 128] natural pairs
            w2pair1 = AP(w2p_f8.tensor, w2p_f8.offset,
                         [w2p_f8.ap[0], (128, 2), (1, 128)])
            w2pair2 = AP(w2p_f8.tensor, w2p_f8.offset + 256,
                         [w2p_f8.ap[0], (128, 2), (1, 128)])
            lw1_sb = cpool.tile([128, HID], f16)
            nc.sync.dma_start(lw1_sb[:, :], lw1_d.ap())
            lw2_sb = cpool.tile([128, CLASSES], f16)
            nc.sync.dma_start(lw2_sb[:, :], lw2_d.ap())
            b1_sb = cpool.tile([128, 1], f32)
            nc.sync.dma_start(b1_sb[:, :], b1_d.ap())
            b2_sb = cpool.tile([128, 1], f32)
            nc.sync.dma_start(b2_sb[:, :], b2_d.ap())
            lb1_sb = cpool.tile([128, 1], f32)
            nc.sync.dma_start(lb1_sb[:, :], lb1_d.ap())
            lb2_sb = cpool.tile([CLASSES, 1], f32)
            nc.sync.dma_start(lb2_sb[:, :], lb2_d.ap())
            idx_sb = cpool.tile([128, nbatch * GCHUNK * ncols], i16)
            nc.sync.dma_start(idx_sb[:, :], idx_d.ap())

            y_sb = cpool.tile([128, nbatch], f16, tag="ytile")

            pending = []  # delayed conv2 blocks, one batch behind
            for b in range(nbatch):
                xts = []
                for c in range(GCHUNK):
                    xt = xt_pool.tile([128, K, CNIDX], f16, tag="xt")
                    o = (b * GCHUNK + c) * ncols
                    nc.gpsimd.dma_gather(
                        out_ap=xt[:, :, :],
                        in_ap=ew_d.ap(),
                        idxs_ap=idx_sb[:, o : o + ncols],
                        num_idxs=CNIDX,
                        num_idxs_reg=NVALID,
                        elem_size=EWC,
                        transpose=True,
                        single_packet=SINGLE_PACKET,
                        queue_num=(b * GCHUNK + c) % NQUEUES,
                    )
                    xts.append(xt)

                # h1: two fp8 copies A at [0, HD) and B at [HD, 2*HD),
                # each col c in [0, L+3): 0,1 wrap; 2..L+2 conv1; L+2 pad
                h1t = h1_pool.tile([128, 2 * HD], u8, tag="h1")
                h1 = h1t[:, :].bitcast(f8)

                mx = sm_pool.tile([128, NLT], f32, tag="mx")

                def conv1(lt):
                    l0 = lt * LTILE
                    xt = xts[l0 // CSPAN]
                    base = l0 % CSPAN
                    ps1 = ps_pool.tile([128, LTILE], f32, tag="ps")
                    # conv1 output = sum of k-shifted gathered slices
                    for k in range(K):
                        nc.tensor.matmul(
                            ps1[:, :],
                            lhsT=id_sb[:, :],
                            rhs=xt[:, k, base + 2 - k : base + 2 - k + LTILE],
                            start=(k == 0),
                            stop=(k == K - 1),
                        )
                    # relu(256*(ps+b1)) -> fp8, dual-written to A and B
                    pin = AP(ps1[:, :].tensor, ps1[:, :].offset,
                             [ps1[:, :].ap[0], (0, 2), (1, LTILE)])
                    pout = AP(h1.tensor, h1.offset + 2 + l0,
                              [h1.ap[0], (HD, 2), (1, LTILE)])
                    nc.scalar.activation(pout, pin, RELU,
                                         bias=b1_sb[:, 0:1], scale=H1S)

                def conv2(lt, h1=h1, mx=mx):
                    l0 = lt * LTILE
                    ps2 = ps_pool.tile([128, LTILE], f32, tag="ps")
                    # pair1: (w2k2 @ A[l0], w2k1 @ B[l0+1]); i-stride HD+1
                    nc.tensor.matmul(
                        ps2[:, :], lhsT=w2pair1,
                        rhs=pair_ap(h1, l0, HD + 1, LTILE),
                        start=True, stop=False, perf_mode=DR)
                    # pair2: (w2k0 @ A[l0+2], 0 @ B[l0+3])
                    nc.tensor.matmul(
                        ps2[:, :], lhsT=w2pair2,
                        rhs=pair_ap(h1, l0 + 2, HD + 1, LTILE),
                        start=False, stop=True, perf_mode=DR)
                    nc.vector.reduce_max(mx[:, lt : lt + 1], ps2[:, :], axis=AX)

                def wrap_cols():
                    # circular wrap cols 0,1 <- cols L,L+1; pad <- col 2
                    for off in (0, HD):
                        nc.scalar.activation(h1[:, off : off + 2],
                                             h1[:, L : L + 2], COPY)
                        nc.scalar.activation(h1[:, off + L + 2 : off + L + 3],
                                             h1[:, 2 : 3], COPY)

                def finish(b=b, mx=mx):
                    pooled = sm_pool.tile([128, 1], f32, tag="pooled")
                    nc.vector.reduce_max(pooled[:, :], mx[:, :], axis=AX)
                    nc.scalar.activation(
                        y_sb[:, b : b + 1], pooled[:, :], RELU,
                        bias=b2_sb[:, 0:1], scale=1.0 / (H1S * W2S))

                # first half of conv1, then the PREVIOUS batch's conv2
                # block (fills PE time while this batch's gathers stream),
                # then the rest of conv1
                for lt in range(0, NLT // 2):
                    conv1(lt)
                if pending:
                    pending.pop(0)()
                if b < nbatch - 1:
                    for lt in range(NLT // 2, NLT):
                        conv1(lt)
                    wrap_cols()

                    def conv2_block(conv2=conv2, fin=finish):
                        for lt in list(range(1, NLT)) + [0]:
                            conv2(lt)
                        fin()

                    pending.append(conv2_block)
                else:
                    # last batch: run conv2 as two grouped sub-blocks woven
                    # between conv1 sub-phases (block-granular, so no
                    # per-tile DR<->normal thrash) to shorten the tail
                    conv1(4)
                    conv1(5)
                    for lt in (1, 2, 3, 4):
                        conv2(lt)
                    conv1(6)
                    conv1(7)
                    wrap_cols()
                    for lt in (5, 6, 7, 0):
                        conv2(lt)
                    finish()

            while pending:
                pending.pop(0)()

            # --- tiny MLP head on all nbatch columns at once ---
            psm1 = psm_pool.tile([128, nbatch], f32, tag="psm1")
            nc.tensor.matmul(psm1[:, :], lhsT=lw1_sb[:, :], rhs=y_sb[:, :],
                             start=True, stop=True)
            z1 = sm_pool.tile([128, nbatch], f16, tag="z1")
            nc.scalar.activation(z1[:, :], psm1[:, :], RELU, bias=lb1_sb[:, 0:1])

            psm2 = psm_pool.tile([CLASSES, nbatch], f32, tag="psm2")
            nc.tensor.matmul(psm2[:, :], lhsT=lw2_sb[:, :], rhs=z1[:, :],
                             start=True, stop=True)
            out_sb = sm_pool.tile([CLASSES, nbatch], f32, tag="osb")
            nc.scalar.activation(out_sb[:, :], psm2[:, :], IDENT,
                                 bias=lb2_sb[:, 0:1])
            nc.sync.dma_start(out_d.ap(), out_sb[:, :])

    nc.compile()
    return nc


def prep_host_inputs(tokens, emb, w1, b1, w2, b2, lw1, lb1, lw2, lb2,
                     mode="f16", nbatch=BLOC):
    """Host-side layout prep.  Returns per-core in_maps."""
    import ml_dtypes

    npdt = _dtype_np(mode)
    tokens = np.asarray(tokens).astype(np.int64)
    emb = np.asarray(emb, np.float32)
    w1 = np.asarray(w1, np.float32)               # [HID, EMB, K]

    # fused conv1 table: ew[t, k*HID + o] = sum_c emb[t, c] * w1[o, c, k]
    ew = np.empty((VOCAB, EWC), np.float32)
    for k in range(K):
        ew[:, k * HID : (k + 1) * HID] = emb @ w1[:, :, k].T
    ew = np.ascontiguousarray(ew.astype(npdt))

    # fp8 DoubleRow lhsT pairs: lhsT[p, i, o] = w2q[o, p, k_i]
    # pair1: (k=2, k=1); pair2: (k=0, zeros)
    w2 = np.asarray(w2, np.float32)               # [HID, HID, K]
    w2q = np.clip(w2 * W2S, -240, 240).astype(ml_dtypes.float8_e4m3)
    w2p = np.zeros((128, 2, 2, 128), ml_dtypes.float8_e4m3)
    w2p[:, 0, 0, :] = w2q[:, :, 2].T              # [p, o]
    w2p[:, 0, 1, :] = w2q[:, :, 1].T
    w2p[:, 1, 0, :] = w2q[:, :, 0].T
    w2p = np.ascontiguousarray(w2p.reshape(128, 512).view(np.uint8))

    lw1t = np.ascontiguousarray(np.asarray(lw1, np.float32).T.astype(npdt))
    lw2t = np.ascontiguousarray(np.asarray(lw2, np.float32).T.astype(npdt))
    b1c = (np.asarray(b1, np.float32) * H1S).reshape(128, 1)
    b2c = np.asarray(b2, np.float32).reshape(128, 1)
    lb1c = np.asarray(lb1, np.float32).reshape(128, 1)
    lb2c = np.asarray(lb2, np.float32).reshape(CLASSES, 1)
    id128 = np.eye(128, dtype=np.float32).astype(npdt)

    pos = np.arange(CNIDX)
    in_maps = []
    for c in range(NCORES):
        idx_cols = []
        for j in range(nbatch):
            t = tokens[c * BLOC + j]
            for g in range(GCHUNK):
                ext = t[(g * CSPAN - 2 + pos) % L].astype(np.int16)
                ext[NVALID:] = -1  # ucode trims trailing -1s
                wrapped = ext.reshape(CNIDX // 16, 16).T
                idx_cols.append(np.tile(wrapped, (8, 1)))      # [128, ncols]
        idx = np.ascontiguousarray(np.concatenate(idx_cols, axis=1))
        in_maps.append({
            "ew": ew, "idx": idx, "id128": id128, "w2p": w2p,
            "b1c": b1c, "b2c": b2c, "lw1t": lw1t, "lb1c": lb1c,
            "lw2t": lw2t, "lb2c": lb2c,
        })
    return in_maps


_CACHE = {}


def _get_program(mode):
    if mode not in _CACHE:
        _CACHE[mode] = build_program(mode)
    return _CACHE[mode]


def run(inputs, mode=None, trace=False, trace_kwargs=None):
    """Run on 8 cores; returns (output[32, 6] f32, BassKernelResults)."""
    from concourse import bass_utils

    mode = mode or os.environ.get("KERNEL_MODE", "f16")
    nc = _get_program(mode)
    in_maps = prep_host_inputs(**inputs, mode=mode)
    res = bass_utils.run_bass_kernel_spmd(
        nc, in_maps, core_ids=list(range(NCORES)), trace=trace,
        **(trace_kwargs or {}),
    )
    out = np.empty((B, CLASSES), np.float32)
    for c in range(NCORES):
        o = res.results[c]["out"]  # [CLASSES, BLOC]
        out[c * BLOC : (c + 1) * BLOC, :] = np.asarray(o, np.float32).T
    return out, res


def kernel(**inputs):
    out, _ = run(inputs)
    return out
